# revision 4
# baseline (speedup 1.0000x reference)
"""LIF spike (vanilla) Trainium2 kernel.

Reference recurrence over leading time dim T (per element):
    u_t = TAU * u_{t-1} * (1 - o_{t-1}) + x_t
    o_t = (u_t - VTH > 0) ? 1.0 : 0.0

Decomposed into 3 elementwise ops per time step on carried state
c = u * (u <= VTH):
    S1: u = (c mult TAU) add x_t        (scalar_tensor_tensor, skipped at t=0)
    S2: o = (u is_gt VTH)               (tensor_scalar -> spike output)
    S3: c = (u is_le VTH) mult u        (scalar_tensor_tensor, skipped at t=T-1)

All compares are exact fp32, so the spike train matches the fp32 jax
reference bit-for-bit. The spike output is stored as uint8 (0/1) to cut
HBM write traffic 4x; the host upcasts to f32. S2 runs on GpSimd so the
DVE only runs the two 2-src ops (which never enter 2-port perf mode and
therefore never contend with GpSimd for the shared SBUF port).

Sharding: pure data parallel over batch dim B=64 -> 8 cores x 8 batches.
Per core: 32MiB in (f32) + 8MiB out (u8) HBM traffic.
"""

import numpy as np

T = 8
B = 64
C = 128
H = 32
W = 32
NCORES = 8
BS = B // NCORES            # batches per core
N = BS * C * H * W          # 1,048,576 elements per time step per core
P = 128                     # SBUF partitions
F = 4096                    # tile free-dim (tile = [128, 4096] f32 = 2MB)
NCHUNK = N // (P * F)       # spatial chunks per core
TAU = 0.5
VTH = 0.99999

OUT_DTYPE = "uint8"         # device-side spike dtype: "float32"|"bfloat16"|"uint8"
S2_ENGINE = "gpsimd"        # engine for the compare: "vector"|"gpsimd"


def _build(nt=T, nchunk=NCHUNK, fdim=F, xb=3, ob=4, ub=2, cb=2,
           out_dtype=OUT_DTYPE, s2_engine=S2_ENGINE):
    import concourse.bacc as bacc
    import concourse.mybir as mybir
    import concourse.tile as tile

    f32 = mybir.dt.float32
    odt = getattr(mybir.dt, out_dtype)
    alu = mybir.AluOpType
    nc = bacc.Bacc("TRN2", target_bir_lowering=False)
    x = nc.dram_tensor("x", [nt, nchunk, P, fdim], f32, kind="ExternalInput")
    o = nc.dram_tensor("o", [nt, nchunk, P, fdim], odt, kind="ExternalOutput")
    with tile.TileContext(nc) as tc:
        s2 = getattr(nc, s2_engine)
        with (
            tc.tile_pool(name="xp", bufs=xb) as xp,
            tc.tile_pool(name="opool", bufs=ob) as opl,
            tc.tile_pool(name="up", bufs=ub) as up,
            tc.tile_pool(name="cp", bufs=cb) as cp,
        ):
            for i in range(nchunk):
                ct = None
                for t in range(nt):
                    xt = xp.tile([P, fdim], f32)
                    nc.sync.dma_start(xt[:], x[t, i])
                    if t == 0:
                        u = xt
                    else:
                        u = up.tile([P, fdim], f32)
                        nc.vector.scalar_tensor_tensor(
                            u[:], ct[:], TAU, xt[:], alu.mult, alu.add
                        )
                    ot = opl.tile([P, fdim], odt)
                    s2.tensor_scalar(ot[:], u[:], VTH, None, alu.is_gt)
                    nc.sync.dma_start(o[t, i], ot[:])
                    if t < nt - 1:
                        ct = cp.tile([P, fdim], f32)
                        nc.vector.scalar_tensor_tensor(
                            ct[:], u[:], VTH, u[:], alu.is_le, alu.mult
                        )
    nc.finalize()
    return nc


def kernel(x):
    x = np.ascontiguousarray(np.asarray(x, dtype=np.float32))
    assert x.shape == (T, B, C, H, W), x.shape
    from concourse.bass_utils import run_bass_kernel_spmd

    nc = _build()
    in_maps = []
    for i in range(NCORES):
        s = np.ascontiguousarray(x[:, i * BS : (i + 1) * BS])
        in_maps.append({"x": s.reshape(T, NCHUNK, P, F)})
    res = run_bass_kernel_spmd(nc, in_maps, core_ids=list(range(NCORES)))
    out = np.empty((T, B, C, H, W), np.float32)
    for i, r in enumerate(res.results):
        o = np.asarray(r["o"]).astype(np.float32)
        out[:, i * BS : (i + 1) * BS] = o.reshape(T, BS, C, H, W)
    return out


# revision 7
# speedup vs baseline: 5.9862x; 5.9862x over previous
"""LIF spike (vanilla) Trainium2 kernel.

Reference recurrence over leading time dim T (per element):
    u_t = TAU * u_{t-1} * (1 - o_{t-1}) + x_t
    o_t = (u_t - VTH > 0) ? 1.0 : 0.0

Decomposed into 3 elementwise ops per time step on carried state
c = u * (u <= VTH):
    S1: u = (c mult TAU) add x_t        (scalar_tensor_tensor, skipped at t=0)
    S2: o = (u is_gt VTH)               (tensor_scalar -> spike output)
    S3: c = (u is_le VTH) mult u        (scalar_tensor_tensor, skipped at t=T-1)

All compares are exact fp32, so the spike train matches the fp32 jax
reference bit-for-bit. The spike output is stored as uint8 (0/1) to cut
HBM write traffic 4x; the host upcasts to f32. S2 runs on GpSimd so the
DVE only runs the two 2-src ops (which never enter 2-port perf mode and
therefore never contend with GpSimd for the shared SBUF port).

Sharding: pure data parallel over batch dim B=64 -> 8 cores x 8 batches.
Per core: 32MiB in (f32) + 8MiB out (u8) HBM traffic.
"""

import numpy as np

T = 8
B = 64
C = 128
H = 32
W = 32
NCORES = 8
BS = B // NCORES            # batches per core
N = BS * C * H * W          # 1,048,576 elements per time step per core
P = 128                     # SBUF partitions
F = 4096                    # tile free-dim (tile = [128, 4096] f32 = 2MB)
NCHUNK = N // (P * F)       # spatial chunks per core
TAU = 0.5
VTH = 0.99999

OUT_DTYPE = "uint8"         # DRAM spike dtype ("float32"|"uint8"); u8 cast in DMA
ACCUM_LOAD = False          # use SWDGE accumulating loads for the S1 add


def _build(nt=T, nchunk=NCHUNK, fdim=F, xb=3, ob=3, ub=2, cb=1,
           out_dtype=OUT_DTYPE, accum_load=ACCUM_LOAD):
    import concourse.bacc as bacc
    import concourse.mybir as mybir
    import concourse.tile as tile

    f32 = mybir.dt.float32
    odt = getattr(mybir.dt, out_dtype)
    alu = mybir.AluOpType
    nc = bacc.Bacc("TRN2", target_bir_lowering=False)
    x = nc.dram_tensor("x", [nt, nchunk, P, fdim], f32, kind="ExternalInput")
    o = nc.dram_tensor("o", [nt, nchunk, P, fdim], odt, kind="ExternalOutput")
    cast_store = out_dtype != "float32"
    with tile.TileContext(nc) as tc:
        with (
            tc.tile_pool(name="xp", bufs=xb) as xp,
            tc.tile_pool(name="opool", bufs=ob) as opl,
            tc.tile_pool(name="up", bufs=ub) as up,
            tc.tile_pool(name="cp", bufs=cb) as cp,
        ):
            for i in range(nchunk):
                ct = None
                for t in range(nt):
                    if t == 0:
                        u = xp.tile([P, fdim], f32)
                        nc.sync.dma_start(u[:], x[t, i])
                    elif accum_load:
                        # u := tau*c, then DMA adds x_t in-flight (CCE add)
                        u = up.tile([P, fdim], f32)
                        nc.vector.tensor_scalar_mul(u[:], ct[:], TAU)
                        nc.gpsimd.dma_start(u[:], x[t, i], accum_op=alu.add)
                    else:
                        xt = xp.tile([P, fdim], f32)
                        nc.sync.dma_start(xt[:], x[t, i])
                        u = up.tile([P, fdim], f32)
                        nc.vector.scalar_tensor_tensor(
                            u[:], ct[:], TAU, xt[:], alu.mult, alu.add
                        )
                    ot = opl.tile([P, fdim], f32)
                    nc.vector.tensor_scalar(ot[:], u[:], VTH, None, alu.is_gt)
                    if cast_store:
                        nc.gpsimd.dma_start(o[t, i], ot[:])
                    else:
                        nc.sync.dma_start(o[t, i], ot[:])
                    if t < nt - 1:
                        ct = cp.tile([P, fdim], f32)
                        nc.vector.scalar_tensor_tensor(
                            ct[:], u[:], VTH, u[:], alu.is_le, alu.mult
                        )
    nc.finalize()
    return nc


def kernel(x):
    x = np.ascontiguousarray(np.asarray(x, dtype=np.float32))
    assert x.shape == (T, B, C, H, W), x.shape
    from concourse.bass_utils import run_bass_kernel_spmd

    nc = _build()
    in_maps = []
    for i in range(NCORES):
        s = np.ascontiguousarray(x[:, i * BS : (i + 1) * BS])
        in_maps.append({"x": s.reshape(T, NCHUNK, P, F)})
    res = run_bass_kernel_spmd(nc, in_maps, core_ids=list(range(NCORES)))
    out = np.empty((T, B, C, H, W), np.float32)
    for i, r in enumerate(res.results):
        o = np.asarray(r["o"]).astype(np.float32)
        out[:, i * BS : (i + 1) * BS] = o.reshape(T, BS, C, H, W)
    return out


# revision 11
# speedup vs baseline: 6.7419x; 1.1262x over previous
"""LIF spike (vanilla) Trainium2 kernel.

Reference recurrence over leading time dim T (per element):
    u_t = TAU * u_{t-1} * (1 - o_{t-1}) + x_t
    o_t = (u_t - VTH > 0) ? 1.0 : 0.0

Decomposed into 3 elementwise ops per time step on carried state
c = u * (u <= VTH):
    S1: u = (c mult TAU) add x_t        (scalar_tensor_tensor, skipped at t=0)
    S2: o = (u is_gt VTH)               (tensor_scalar -> spike output)
    S3: c = (u is_le VTH) mult u        (scalar_tensor_tensor, skipped at t=T-1)

All compares are exact fp32, so the spike train matches the fp32 jax
reference bit-for-bit. The spike output is cast to uint8 (0/1) in the
store DMA (SWDGE) to cut HBM write traffic 4x; the host upcasts to f32.

Sharding: pure data parallel over batch dim B=64 -> 8 cores x 8 batches.
Per core: 32MiB in (f32) + 8MiB out (u8) HBM traffic.
"""

import numpy as np

T = 8
B = 64
C = 128
H = 32
W = 32
NCORES = 8
BS = B // NCORES            # batches per core
N = BS * C * H * W          # 1,048,576 elements per time step per core
P = 128                     # SBUF partitions
F = 4096                    # tile free-dim (tile = [128, 4096] f32 = 2MB)
NCHUNK = N // (P * F)       # spatial chunks per core
TAU = 0.5
VTH = 0.99999

OUT_DTYPE = "bfloat16"      # DRAM spike repr: relu(u-VTH) in bf16; host maps >0 -> 1.0
ACCUM_LOAD = False          # SWDGE accumulating loads (broken at runtime; keep off)


def _build(nt=T, nchunk=NCHUNK, fdim=F, xb=3, ob=3, ub=2, cb=1,
           out_dtype=OUT_DTYPE, accum_load=ACCUM_LOAD):
    import concourse.bacc as bacc
    import concourse.mybir as mybir
    import concourse.tile as tile

    f32 = mybir.dt.float32
    odt = getattr(mybir.dt, out_dtype)
    alu = mybir.AluOpType
    nc = bacc.Bacc("TRN2", target_bir_lowering=False)
    x = nc.dram_tensor("x", [nt, nchunk, P, fdim], f32, kind="ExternalInput")
    o = nc.dram_tensor("o", [nt, nchunk, P, fdim], odt, kind="ExternalOutput")
    s2_act = out_dtype == "bfloat16"
    with tile.TileContext(nc) as tc:
        with (
            tc.tile_pool(name="const", bufs=1) as constp,
            tc.tile_pool(name="xp", bufs=xb) as xp,
            tc.tile_pool(name="opool", bufs=ob) as opl,
            tc.tile_pool(name="up", bufs=ub) as up,
            tc.tile_pool(name="cp", bufs=cb) as cp,
        ):
            nvth = constp.tile([P, 1], f32)
            nc.vector.memset(nvth[:], -VTH)
            for i in range(nchunk):
                ct = None
                for t in range(nt):
                    if t == 0:
                        u = xp.tile([P, fdim], f32)
                        nc.sync.dma_start(u[:], x[t, i])
                    elif accum_load:
                        # u := tau*c, then DMA adds x_t in-flight (CCE add)
                        u = up.tile([P, fdim], f32)
                        nc.vector.tensor_scalar_mul(u[:], ct[:], TAU)
                        nc.gpsimd.dma_start(u[:], x[t, i], accum_op=alu.add)
                    else:
                        xt = xp.tile([P, fdim], f32)
                        nc.sync.dma_start(xt[:], x[t, i])
                        u = up.tile([P, fdim], f32)
                        nc.vector.scalar_tensor_tensor(
                            u[:], ct[:], TAU, xt[:], alu.mult, alu.add
                        )
                    ot = opl.tile([P, fdim], odt)
                    if s2_act:
                        # spike iff relu(u - VTH) > 0; exact in fp32, and any
                        # positive fp32 survives the bf16 downcast as positive
                        nc.scalar.activation(
                            ot[:], u[:], mybir.ActivationFunctionType.Relu,
                            bias=nvth[:], scale=1.0,
                        )
                    else:
                        nc.vector.tensor_scalar(ot[:], u[:], VTH, None, alu.is_gt)
                    nc.sync.dma_start(o[t, i], ot[:])
                    if t < nt - 1:
                        ct = cp.tile([P, fdim], f32)
                        nc.vector.scalar_tensor_tensor(
                            ct[:], u[:], VTH, u[:], alu.is_le, alu.mult
                        )
    nc.finalize()
    return nc


def kernel(x):
    x = np.ascontiguousarray(np.asarray(x, dtype=np.float32))
    assert x.shape == (T, B, C, H, W), x.shape
    from concourse.bass_utils import run_bass_kernel_spmd

    nc = _build()
    in_maps = []
    for i in range(NCORES):
        s = np.ascontiguousarray(x[:, i * BS : (i + 1) * BS])
        in_maps.append({"x": s.reshape(T, NCHUNK, P, F)})
    res = run_bass_kernel_spmd(nc, in_maps, core_ids=list(range(NCORES)))
    out = np.empty((T, B, C, H, W), np.float32)
    for i, r in enumerate(res.results):
        out[:, i * BS : (i + 1) * BS] = _decode(r["o"]).reshape(T, BS, C, H, W)
    return out


def _decode(o):
    """Device spike repr -> f32 spike train (bf16 relu(u-VTH): spike iff >0)."""
    o = np.asarray(o)
    if o.dtype == np.float32:
        return o
    return (o > 0).astype(np.float32)
